# revision 24
# baseline (speedup 1.0000x reference)
"""Per-frame RMS energy (STFT framing: n_fft=1024, hop=256, center/reflect pad)
over a [16, 1048576] f32 signal -> [16, 4096, 1] f32.

Trainium2 Bass/Tile kernel, data-parallel over batch across 8 NeuronCores
(2 signals per core). Each 1024-sample frame is exactly 4 consecutive
256-sample hop blocks, so we compute per-block sums of squares (one read of
every input byte -> memory-bound optimal), then a sliding sum of 4 plus
sqrt(mean).

Layout: partition p of a signal owns frames p*32..p*32+31; its input row is
the naturally aligned x[p*8192 : (p+1)*8192] (fast full-128-partition
power-of-2 DMAs). ext[p, u] = s_pad[p*32+u] (u in 0..34) where s_pad[b] is
the padded-signal 256-block sum of squares; cols 2..33 come straight from
the grouped reduces, the 3-value seam from the neighbor partition comes via
two tiny SBUF->SBUF DMAs, and the reflect-pad edge sums via fused
square+accumulate on the scalar engine.

Engine plan: Sync HWDGE ring carries ONLY the bulk load stream (so it is
never head-of-line blocked), ACT squares + edge sums + final sqrt + output
DMAs (its own HWDGE ring), DVE does all 256-block reduces + window adds,
GpSimd SWDGE carries the tiny edge loads and seam copies. Seam-source
chunks are processed first so seam DMAs complete during the stream.
"""

import sys
import types

import numpy as np

import concourse.bacc as bacc
import concourse.mybir as mybir
import concourse.tile as tile
from concourse.bass_utils import run_bass_kernel_spmd
from concourse.vector_clock import ScopedClock


def _install_ntff_hook_shim():
    """The image's antenv lacks axon_hooks; if a caller turns on tracing
    (e.g. via BASS_TRACE=1), run_bass_kernel_spmd imports it. Provide the
    ctypes-based hook so that path works instead of raising."""
    try:
        import antenv.axon_hooks  # noqa: F401

        return
    except ImportError:
        pass
    try:
        from trn_agent_boot.trn_boot import _ntff_profile_via_ctypes

        hook = _ntff_profile_via_ctypes("/opt/axon/libaxon_pjrt.so")
    except Exception:
        hook = None
    mod = types.ModuleType("antenv.axon_hooks")
    mod.get_axon_ntff_profile_hook = lambda: hook
    mod.set_axon_ntff_profile_hook = lambda h: None
    sys.modules["antenv.axon_hooks"] = mod


_install_ntff_hook_shim()


class SlimExitTileContext(tile.TileContext):
    """TileContext whose exit sequence drops the second all-engine barrier.

    The stock epilogue is drain -> barrier -> sem clear -> barrier. The
    first barrier guarantees every engine is idle before the gpsimd range
    sem-clear runs; the trailing barrier only re-synchronizes engines that
    are each about to run off the end of their own queues, so skipping it
    is safe (NRT completion still waits for every queue, and the sem state
    a re-execution needs is restored by the clear).
    """

    def _drain_and_barrier(self, tick_clock, wait_clock):
        drain_inst = self.nc.sync.drain()
        wait_clock.add_sem_waits(
            drain_inst.ins, ScopedClock({None: tick_clock.global_clock})
        )
        self.nc.all_engine_barrier(sem_only=True)
        assert self.sems is not None
        popped = self.nc._tile_sem_poison_stack.pop()
        assert popped is self._sem_poison
        self.nc.clear_and_free_semaphores(list(self.sems.allocated().values()))

# Problem constants (self-contained; must match the grader's input spec)
B = 16                 # signals in the batch
T = 1048576            # samples per signal
N_FFT = 1024
HOP = 256
N_CORES = 8
SIG_PER_CORE = B // N_CORES   # 2
P = 128                       # SBUF partitions
NBLK = T // HOP               # 4096 hop blocks per signal
CPB = NBLK // P               # 32 output frames per partition
SPP = T // P                  # 8192 samples per partition row
NFRAMES = NBLK                # 4096 output frames per signal

# Per-signal chunks of the 8192-sample partition row, in 256-blocks
# (block_offset, n_blocks). Seam-source chunks (last 2 blocks / first block)
# first; small chunks at the end shorten the post-stream tail.
CHUNKS = [(28, 4), (0, 4), (4, 4), (8, 4), (12, 4), (16, 4), (20, 4), (24, 2), (26, 2)]

F32 = mybir.dt.float32
AF = mybir.ActivationFunctionType
AX = mybir.AxisListType
ADD = mybir.AluOpType.add


def build_bass():
    # Bacc (not plain Bass): its compile pipeline splits multi-sem waits into
    # event-semaphore instructions, which this walrus build requires.
    nc = bacc.Bacc()
    x = nc.dram_tensor("signal", [SIG_PER_CORE, T], F32, kind="ExternalInput")
    y = nc.dram_tensor("out", [SIG_PER_CORE, NFRAMES], F32, kind="ExternalOutput")

    xr = x[:, :].rearrange("b (p f) -> b p f", p=P)   # [2, 128, 8192]
    yr = y[:, :].rearrange("b (p c) -> b p c", p=P)   # [2, 128, 32]

    with SlimExitTileContext(nc) as tc:
        with (
            tc.tile_pool(name="inp", bufs=8) as inp_pool,
            tc.tile_pool(name="sq", bufs=6) as sq_pool,
            tc.tile_pool(name="ext", bufs=2) as ext_pool,
            tc.tile_pool(name="spec", bufs=2) as spec_pool,
            tc.tile_pool(name="small", bufs=2) as small_pool,
        ):
            exts = []
            # Dummy Sqrt first so the ACT table set that covers both Square
            # and Sqrt loads once, up front, in the preamble shadow (instead
            # of a ~1.3us reload injected mid-stream before the first real
            # sqrt).
            dummy = spec_pool.tile([1, 1], F32, tag="dummy")
            nc.vector.memset(dummy[0:1, 0:1], 1.0)
            nc.scalar.activation(
                out=dummy[0:1, 0:1], in_=dummy[0:1, 0:1], func=AF.Sqrt
            )
            # Phase 0: tiny edge loads for both signals (SWDGE, overlap the
            # stream). spc row 0 = [x[1:257], x[257:513], x[T-257:T-1]].
            spcs = []
            for sig in range(SIG_PER_CORE):
                spc = spec_pool.tile([P, 768], F32, tag="spc")
                nc.gpsimd.dma_start(out=spc[0:1, 0:512], in_=x[sig : sig + 1, 1:513])
                nc.gpsimd.dma_start(
                    out=spc[0:1, 512:768], in_=x[sig : sig + 1, T - 257 : T - 1]
                )
                spcs.append(spc)

            # Phase 1: per signal, stream chunks (load -> square -> block
            # reduce), seam copies, then edge sums on ACT.
            for sig in range(SIG_PER_CORE):
                ext = ext_pool.tile([P, 36], F32)
                exts.append(ext)
                for ci, (b0, nb) in enumerate(CHUNKS):
                    ln = nb * HOP
                    tin = inp_pool.tile([P, ln], F32, tag="tin")
                    nc.sync.dma_start(
                        out=tin[:, :],
                        in_=xr[sig, :, b0 * HOP : b0 * HOP + ln],
                    )
                    tsq = sq_pool.tile([P, ln], F32, tag="tsq")
                    nc.scalar.activation(out=tsq[:, :], in_=tin[:, :], func=AF.Square)
                    nc.vector.tensor_reduce(
                        out=ext[:, 2 + b0 : 2 + b0 + nb],
                        in_=tsq[:, :].rearrange("p (g k) -> p g k", k=HOP),
                        axis=AX.X,
                        op=ADD,
                    )
                    # Cross-partition seam moves, as early as their sources
                    # exist (chunk order puts those sources first):
                    if ci == 0:
                        # ext[p, 0:2] = s_pad[p*32 .. +1] = ext[p-1, 32:34]
                        nc.gpsimd.dma_start(
                            out=ext[1:128, 0:2], in_=ext[0:127, 32:34]
                        )
                    elif ci == 1:
                        # ext[p, 34] = s_pad[p*32+34] = ext[p+1, 2]
                        nc.gpsimd.dma_start(
                            out=ext[0:127, 34:35], in_=ext[1:128, 2:3]
                        )

                # Reflect-pad edge sums, fused square+accumulate on ACT:
                #   s_pad[1]    = sum x[1:257]^2     -> ext[0, 1]
                #   s_pad[0]    = sum x[257:513]^2   -> ext[0, 0]
                #   s_pad[4098] = sum x[T-257:T-1]^2 -> ext[127, 34] (via spr)
                spc = spcs[sig]
                spq = spec_pool.tile([P, 768], F32, tag="spq")
                spr = spec_pool.tile([P, 1], F32, tag="spr")
                nc.scalar.activation(
                    out=spq[0:1, 0:256], in_=spc[0:1, 0:256], func=AF.Square,
                    accum_out=ext[0:1, 1:2],
                )
                nc.scalar.activation(
                    out=spq[0:1, 256:512], in_=spc[0:1, 256:512], func=AF.Square,
                    accum_out=ext[0:1, 0:1],
                )
                nc.scalar.activation(
                    out=spq[0:1, 512:768], in_=spc[0:1, 512:768], func=AF.Square,
                    accum_out=spr[0:1, 0:1],
                )
                nc.gpsimd.dma_start(out=ext[127:128, 34:35], in_=spr[0:1, 0:1])

            # Phase 2: window-of-4 sums + sqrt(mean) + output, per signal.
            # E[p, c] = ext[p, c] + ext[p, c+1] + ext[p, c+2] + ext[p, c+3]
            for sig in range(SIG_PER_CORE):
                ext = exts[sig]
                e1 = small_pool.tile([P, CPB], F32, tag="e1")
                e2 = small_pool.tile([P, CPB], F32, tag="e2")
                nc.vector.tensor_add(out=e1[:, :], in0=ext[:, 0:32], in1=ext[:, 1:33])
                nc.vector.tensor_add(out=e2[:, :], in0=ext[:, 2:34], in1=ext[:, 3:35])
                nc.vector.tensor_add(out=e1[:, :], in0=e1[:, :], in1=e2[:, :])
                ot = small_pool.tile([P, CPB], F32, tag="ot")
                nc.scalar.activation(
                    out=ot[:, :], in_=e1[:, :], func=AF.Sqrt, scale=1.0 / N_FFT
                )
                nc.scalar.dma_start(out=yr[sig, :, :], in_=ot[:, :])
    nc.finalize()
    return nc


_NC = None


def run(signal: np.ndarray, trace: bool = False):
    global _NC
    sig = np.ascontiguousarray(np.asarray(signal, dtype=np.float32))
    assert sig.shape == (B, T), sig.shape
    if _NC is None:
        _NC = build_bass()
    in_maps = [
        {"signal": np.ascontiguousarray(sig[k * SIG_PER_CORE : (k + 1) * SIG_PER_CORE])}
        for k in range(N_CORES)
    ]
    res = run_bass_kernel_spmd(_NC, in_maps, core_ids=list(range(N_CORES)), trace=trace)
    out = np.concatenate([r["out"] for r in res.results], axis=0)
    return out.reshape(B, NFRAMES, 1).astype(np.float32), res


def kernel(signal: np.ndarray) -> np.ndarray:
    out, _ = run(signal, trace=False)
    return out
